# revision 13
# baseline (speedup 1.0000x reference)
"""AttentionPooling Bass kernel for 8 TRN2 NeuronCores (v2).

Problem: x [262144, 1024] f32, bags of 128 consecutive rows (2048 bags).
  scores = (tanh(x @ W1 + b1) @ W2 + b2)[:, 0]        per-row MLP score
  w      = softmax(scores) within each bag
  out[b] = sum_i w[i] * x[i]  over the bag's rows  -> [2048, 1024] f32

Sharding: data-parallel over bags; core c gets bags [c*256, (c+1)*256).
Weights replicated. No cross-core communication. b2 is dropped (uniform
shift inside each bag's softmax — a no-op for the output).

v2 changes vs v1 (measured 1.286 ms):
- Softmax uses matmul reductions instead of PE transposes + DVE chain:
  denom[1,8] = ones[128,1]^T @ exp(scores[128,8]); the reciprocal is
  broadcast back over partitions with a K=1 matmul. No max-subtraction:
  |scores| <= ~3 (tanh output dotted with W2), exp is safe in f32.
- The W2 dot runs as one fused DVE tensor_tensor_reduce (mult+add)
  instead of separate mul + reduce (saves ~1.2 us/bag of DVE time and
  unjams the DVE queue that stalled the PE at group boundaries).
- Softmax + phase-2 of group g are interleaved INTO group g+1's bag
  loop (stage per bag index) so the PE queue never waits on the
  cross-engine softmax chain; this also stops the HAM clock-gate from
  re-throttling the PE at group boundaries (~137 us cold time in v1).
- Input DMA batched 2 bags per descriptor; output rows written with one
  partition-strided DMA per 4-bag ys tile.
"""

import sys

if "/opt/trn_rl_repo" not in sys.path:
    sys.path.insert(0, "/opt/trn_rl_repo")

import numpy as np

import concourse.bass as bass
import concourse.bacc as bacc
import concourse.mybir as mybir
import concourse.tile as tile
from concourse.bass_utils import run_bass_kernel_spmd
from concourse.masks import make_identity

F32 = mybir.dt.float32
BF16 = mybir.dt.bfloat16
AF = mybir.ActivationFunctionType
ALU = mybir.AluOpType

N_CORES = 8
BAG = 128
D = 1024
H = 1024
DC = D // 128  # contraction chunks
GROUP = 8      # bags per softmax group
WG = 4         # bags per weighted-sum subgroup (PSUM col-group packing)

# set by test.py for profiling; the grading harness leaves these alone
TRACE = False
LAST_EXEC_NS = None
LAST_PROFILE = None
LAST_RESULT = None

_cache = {}


def _build(bags_core: int, with_b1: bool, n_cores: int = N_CORES):
    """Build the per-core Bass module. All cores run the same NEFF."""
    assert bags_core % GROUP == 0 and GROUP % WG == 0
    n_groups = bags_core // GROUP

    nc = bacc.Bacc("TRN2", target_bir_lowering=False, debug=False,
                   num_devices=n_cores)
    x_h = nc.declare_dram_parameter("x", [bags_core * BAG, D], F32,
                                    isOutput=False)
    w1_h = nc.declare_dram_parameter("w1", [D, H], F32, isOutput=False)
    w2_h = nc.declare_dram_parameter("w2", [1, H], F32, isOutput=False)
    b1_h = nc.declare_dram_parameter("b1", [1, H], F32, isOutput=False)
    out_h = nc.declare_dram_parameter("out", [bags_core, D], F32,
                                      isOutput=True)

    with tile.TileContext(nc) as tc:
        with (
            tc.tile_pool(name="const", bufs=1) as const_pool,
            tc.tile_pool(name="xstage", bufs=3) as xs_pool,
            tc.tile_pool(name="xb", bufs=2 * GROUP + 4) as xb_pool,
            tc.tile_pool(name="xt", bufs=3) as xt_pool,
            tc.tile_pool(name="tanh", bufs=2) as t_pool,
            tc.tile_pool(name="dump", bufs=2) as dump_pool,
            tc.tile_pool(name="sink", bufs=2) as sink_pool,
            tc.tile_pool(name="scores", bufs=3) as sc_pool,
            tc.tile_pool(name="soft", bufs=2) as soft_pool,
            tc.tile_pool(name="ystage", bufs=2) as y_pool,
            tc.tile_pool(name="ps_xt", bufs=2, space="PSUM") as ps_xt_pool,
            tc.tile_pool(name="ps_s", bufs=2, space="PSUM") as ps_s_pool,
            tc.tile_pool(name="ps_y", bufs=2, space="PSUM") as ps_y_pool,
            tc.tile_pool(name="ps_sm", bufs=2, space="PSUM") as ps_sm_pool,
        ):
            # ---- constants / weights (resident) ----
            ident_b = const_pool.tile([128, 128], BF16)
            make_identity(nc, ident_b)

            w1_sb = const_pool.tile([128, DC, H], BF16)
            for c in range(DC):
                nc.gpsimd.dma_start(out=w1_sb[:, c, :],
                                    in_=w1_h[c * 128:(c + 1) * 128, :])

            w2_row = const_pool.tile([1, H], BF16)
            nc.gpsimd.dma_start(out=w2_row[:, :], in_=w2_h[:, :])
            ones_row = const_pool.tile([1, 128], BF16)
            nc.any.memset(ones_row[:, :], 1.0)
            ones_row_f = const_pool.tile([1, 128], F32)
            nc.any.memset(ones_row_f[:, :], 1.0)
            ones_col_f = const_pool.tile([128, 1], F32)
            nc.any.memset(ones_col_f[:, :], 1.0)
            # replicate W2 across partitions: ones[1,128].T @ w2_row[1,512]
            w2_rep = const_pool.tile([128, H], BF16)
            for j in range(2):
                ps = ps_s_pool.tile([128, 512], F32, name="ps_s")
                nc.tensor.matmul(ps[:, :], lhsT=ones_row[:, :],
                                 rhs=w2_row[:, 512 * j:512 * (j + 1)],
                                 start=True, stop=True)
                nc.vector.tensor_copy(w2_rep[:, 512 * j:512 * (j + 1)],
                                      ps[:, :])

            if with_b1:
                b1_row = const_pool.tile([1, H], BF16)
                nc.gpsimd.dma_start(out=b1_row[:, :], in_=b1_h[:, :])

            # per-group live state
            xbs_of = {}    # g -> list of x_b tiles
            sc_of = {}     # g -> score tile [128, GROUP] f32
            e_of = {}      # g -> exp tile
            rcp_of = {}    # g -> reciprocal row [1, GROUP]
            wc_of = {}     # g -> weight columns [128, GROUP] bf16

            def phase1_pair(g, n):
                """Load + score two bags (n, n+1) of group g."""
                sc_tile = sc_of[g]
                x_s = xs_pool.tile([128, 2, D], F32)
                bag = g * GROUP + n
                nc.sync.dma_start(
                    out=x_s[:, :, :],
                    in_=x_h[bag * BAG:(bag + 2) * BAG, :].rearrange(
                        "(two p) d -> p two d", two=2))
                x_b2 = xb_pool.tile([128, 2, D], BF16)
                nc.vector.tensor_copy(x_b2[:, :, :], x_s[:, :, :])
                for k in range(2):
                    x_b = x_b2[:, k, :]
                    xbs_of[g].append(x_b2)

                    ps_xt = ps_xt_pool.tile([128, DC, 128], BF16)
                    for c in range(DC):
                        nc.tensor.transpose(ps_xt[:, c, :],
                                            x_b[:, c * 128:(c + 1) * 128],
                                            ident_b[:, :])
                    xt_sb = xt_pool.tile([128, DC, 128], BF16)
                    nc.vector.tensor_copy(xt_sb[:, :, :], ps_xt[:, :, :])

                    t_t = t_pool.tile([128, H], BF16)
                    for j in range(2):
                        ps_s = ps_s_pool.tile([128, 512], F32)
                        for c in range(DC):
                            nc.tensor.matmul(
                                ps_s[:, :],
                                lhsT=xt_sb[:, c, :],
                                rhs=w1_sb[:, c, 512 * j:512 * (j + 1)],
                                start=(c == 0),
                                stop=(c == DC - 1 and not with_b1))
                        if with_b1:
                            nc.tensor.matmul(ps_s[:, :], lhsT=ones_row[:, :],
                                             rhs=b1_row[:, 512 * j:512 * (j + 1)],
                                             start=False, stop=True)
                        nc.scalar.activation(t_t[:, 512 * j:512 * (j + 1)],
                                             ps_s[:, :], AF.Tanh)

                    # W2 dot: DVE multiply, then ScalarE sums via the
                    # activation accumulator (keeps the reduce off the DVE,
                    # whose queue congestion stalled the PE in v1).
                    dump = dump_pool.tile([128, H], BF16)
                    nc.vector.tensor_mul(dump[:, :], t_t[:, :], w2_rep[:, :])
                    sink = sink_pool.tile([128, H], BF16)
                    nc.scalar.activation(sink[:, :], dump[:, :], AF.Copy,
                                         accum_out=sc_tile[:, n + k:n + k + 1])

            def sm_exp(g):
                e_t = soft_pool.tile([128, GROUP], F32, tag="e")
                nc.scalar.activation(e_t[:, :], sc_of[g][:, :], AF.Exp)
                e_of[g] = e_t

            def sm_denom(g):
                ps_d = ps_sm_pool.tile([128, GROUP], F32, name="ps_sm")[0:1, :]
                nc.tensor.matmul(ps_d[:, :], lhsT=ones_col_f[:, :],
                                 rhs=e_of[g][:, :], start=True, stop=True)
                rcp = soft_pool.tile([1, GROUP], F32, tag="r")
                nc.vector.reciprocal(rcp[:, :], ps_d[:, :])
                rcp_of[g] = rcp

            def sm_weights(g):
                ps_b = ps_sm_pool.tile([128, GROUP], F32, name="ps_sm")
                nc.tensor.matmul(ps_b[:, :], lhsT=ones_row_f[:, :],
                                 rhs=rcp_of[g][:, :], start=True, stop=True)
                w_cols = soft_pool.tile([128, GROUP], BF16, tag="w")
                nc.vector.tensor_mul(w_cols[:, :], e_of[g][:, :], ps_b[:, :])
                wc_of[g] = w_cols

            def phase2(g, q):
                """Weighted sums for bags [q*WG, (q+1)*WG) of group g."""
                w_cols = wc_of[g]
                xbs = xbs_of[g]
                ys = y_pool.tile([128, D], F32)
                for j in range(2):
                    ps_y = ps_y_pool.tile([128, 512], F32)
                    for v in range(WG):
                        b = q * WG + v
                        xb2 = xbs[b]
                        nc.tensor.matmul(ps_y[32 * v:32 * v + 1, :],
                                         lhsT=w_cols[:, b:b + 1],
                                         rhs=xb2[:, b % 2,
                                                 512 * j:512 * (j + 1)],
                                         start=True, stop=True,
                                         tile_position=(0, 32 * v))
                    # NOTE: nc.scalar.copy here hangs the device; VectorE ok.
                    nc.vector.tensor_copy(ys[:, 512 * j:512 * (j + 1)],
                                          ps_y[:, :])
                bag0 = g * GROUP + q * WG
                nc.sync.dma_start(out=out_h[bag0:bag0 + WG, :],
                                  in_=ys[0:128:32, :])

            def softmax_stage(g, n):
                """Emit softmax/phase-2 stage n of group g."""
                if n == 0:
                    sm_exp(g)
                elif n == 2:
                    sm_denom(g)
                elif n == 4:
                    sm_weights(g)
                elif n == 6:
                    phase2(g, 0)
                elif n == 7:
                    phase2(g, 1)
                    # release group state
                    for dct in (xbs_of, sc_of, e_of, rcp_of, wc_of):
                        dct.pop(g, None)

            for g in range(n_groups):
                xbs_of[g] = []
                sc_of[g] = sc_pool.tile([128, GROUP], F32, name="sc")
                for n in range(GROUP):
                    if n % 2 == 0:
                        phase1_pair(g, n)
                    if g > 0:
                        softmax_stage(g - 1, n)
            for n in range(GROUP):
                softmax_stage(n_groups - 1, n)

    nc.finalize()
    return nc


def _numpy_fallback(x, W1, b1, W2, b2, bag_sizes):
    seg_ends = np.cumsum(bag_sizes)
    seg_starts = seg_ends - bag_sizes
    scores = (np.tanh(x @ W1 + b1) @ W2 + b2)[:, 0]
    out = np.zeros((bag_sizes.shape[0], x.shape[1]), dtype=x.dtype)
    for i, (s, e) in enumerate(zip(seg_starts, seg_ends)):
        sc = scores[s:e]
        w = np.exp(sc - sc.max())
        w /= w.sum()
        out[i] = w @ x[s:e]
    return out


def kernel(x, W1, b1, W2, b2, bag_sizes):
    x = np.ascontiguousarray(np.asarray(x, dtype=np.float32))
    W1 = np.ascontiguousarray(np.asarray(W1, dtype=np.float32))
    b1 = np.asarray(b1, dtype=np.float32)
    W2 = np.asarray(W2, dtype=np.float32)
    b2 = np.asarray(b2, dtype=np.float32)
    bag_sizes = np.asarray(bag_sizes)

    n_bags = bag_sizes.shape[0]
    if not (np.all(bag_sizes == BAG) and x.shape[0] == n_bags * BAG
            and x.shape[1] == D and n_bags % (N_CORES * GROUP) == 0):
        return _numpy_fallback(x, W1, b1, W2, b2, bag_sizes)

    bags_core = n_bags // N_CORES
    rows_core = bags_core * BAG
    with_b1 = bool(np.any(b1))

    key = (bags_core, with_b1)
    if key not in _cache:
        _cache[key] = _build(bags_core, with_b1)
    nc = _cache[key]

    w2_row = np.ascontiguousarray(W2.reshape(1, H))
    b1_row = np.ascontiguousarray(b1.reshape(1, H))
    in_maps = []
    for c in range(N_CORES):
        in_maps.append({
            "x": x[c * rows_core:(c + 1) * rows_core],
            "w1": W1,
            "w2": w2_row,
            "b1": b1_row,
        })

    res = run_bass_kernel_spmd(nc, in_maps, core_ids=list(range(N_CORES)),
                               trace=TRACE)
    global LAST_EXEC_NS, LAST_PROFILE, LAST_RESULT
    LAST_EXEC_NS = res.exec_time_ns
    LAST_PROFILE = res.profile_json
    LAST_RESULT = res

    return np.concatenate([res.results[c]["out"] for c in range(N_CORES)],
                          axis=0)
